# revision 15
# baseline (speedup 1.0000x reference)
"""Trainium2 Bass kernel for the Kruskal (CP/Tucker) linear layer.

Math: the reference reconstructs W (4096x4096) from a rank-16 CP core and
Tucker factors, then computes y = x @ W.T + bias.  Because the 6D core is a
CP (Kruskal) tensor of rank 16, W itself is exactly rank 16:

    W = g_out @ g_in.T
    g_in[def, r]  = (f3@c3)[d,r] * (f4@c4)[e,r] * (f5@c5)[f,r]   (4096 x 16)
    g_out[abc, r] = (f0@c0)[a,r] * (f1@c1)[b,r] * (f2@c2)[c,r]   (4096 x 16)

so  y = (x @ g_in) @ g_out.T + bias.  The device kernel computes the two
x-dependent projections; the tiny factor-only products (g_in/g_out, ~100
KFLOP) are prepared on the host.

bf16 is used end-to-end (not fp16: the factor entries ~2e-5 and outputs
~1e-6 underflow fp16's 6e-5 min-normal, and fp16 moving operands stream at
half rate on the PE).  All device I/O is bf16, halving HBM and host<->device
traffic vs fp32.

Sharding: data-parallel over the batch (4096 rows -> 8 cores x 512). No
collectives.  x is pre-transposed on the host into the exact SBUF layout
(xT[p, h, kt, b] = x[h*128+b, kt*128+p]) so the device does plain fully
contiguous loads: an on-device xbar DMA-transpose would serialize ALL other
DMA traffic around it (Tile's deadlock guard chains every DMA before/after
a DMA_TRANSPOSE on completion semaphores) and runs at only ~224 GB/s.

Device schedule (per core, 4 pipelined 128-row chunks):
  0. scratch memzero first (no deps), tiny consts (aux/gin/gout) next on
     the scalar HWDGE ring so they are not starved behind the x stream;
     x chunk loads alternate the sync ring and the gpsimd (SWDGE) path so
     chunk 0 shares bandwidth 2-ways instead of 4-ways
  1. PE warm-up: the memzero'd scratch feeds ~14 dummy matmuls during the
     first x load so the HAM clock-gate reaches 8/8 (2.4 GHz) before real
     work; a cold PE runs every matmul at half clock
  2. stage 1: 32 accumulating matmuls  t^T(16,128) += g_in_kt.T @ xT_kt
     (one aux K=1 matmul first writes the bias ones-row 16)
  3. DVE copy PSUM -> SBUF bf16 t^T
  4. stage 2: 8 matmuls  y(128,512) = [t,1].T @ [g_out.T; bias] into 2-bank
     PSUM tiles, evacuated 1024 cols per copy, split DVE/ACT
  5. y stored bf16 per evacuated piece, alternating sync/scalar rings; the
     final chunk stores 512-col pieces to shorten the kernel tail
"""

import numpy as np
import ml_dtypes

N_CORES = 8
BATCH = 4096
D = 4096          # in/out features (16*16*16)
R = 16            # CP rank
P = 128           # partitions
NB = BATCH // N_CORES   # 512 batch rows per core
HB = 128                # batch chunk (rows per pipelined chunk)
NH = NB // HB           # 4 chunks per core
KT = D // P             # 32 feature k-tiles
NT = 512                # output column tile (one PSUM bank)
JT = D // NT            # 8 output column tiles
N_WARM = 26             # dummy matmuls to warm the PE clock-gate

_PROGRAM = None


def _build_program():
    import concourse.tile as tile
    from concourse import bacc, mybir

    nc = bacc.Bacc(
        "TRN2",
        target_bir_lowering=False,
        debug=False,
        enable_asserts=False,
        num_devices=N_CORES,
    )
    bf16 = mybir.dt.bfloat16
    # xc holds x^T in SBUF layout: xc[p, (h*KT + kt)*HB + b] = x[h*HB+b, kt*128+p]
    x_d = nc.dram_tensor("xc", (P, NH * KT * HB), bf16, kind="ExternalInput")
    gin_d = nc.dram_tensor("gin", (P, KT * R), bf16, kind="ExternalInput")
    gout_d = nc.dram_tensor("goutT", (R + 1, D), bf16, kind="ExternalInput")
    # aux row: [e16 (17 cols: zeros, col16=1), ones (HB cols)] used to write
    # the bias ones-row of t^T via a K=1 matmul
    aux_d = nc.dram_tensor("aux", (1, R + 1 + HB), bf16, kind="ExternalInput")
    y_d = nc.dram_tensor("yc", (NB, D), bf16, kind="ExternalOutput")

    with tile.TileContext(nc) as tc:
        with (
            tc.tile_pool(name="const", bufs=1) as constp,
            tc.tile_pool(name="xT", bufs=3) as xTp,
            tc.tile_pool(name="tsb", bufs=2) as tsbp,
            tc.tile_pool(name="ysb", bufs=3) as ysbp,
            tc.tile_pool(name="tpsum", bufs=2, space="PSUM") as tpsump,
            tc.tile_pool(name="ypsum", bufs=2, space="PSUM") as ypsump,
            tc.tile_pool(name="wpsum", bufs=1, space="PSUM") as wpsump,
        ):
            # scratch memzero first (no dependencies) so the PE warm-up can
            # begin as early as possible
            warm_sb = constp.tile([P, NT], bf16)
            nc.scalar.memzero(warm_sb[:])

            # tiny consts next on the scalar ring: they must not queue
            # behind the 4MB x stream (a late aux packet gates the first MM)
            aux_sb = constp.tile([1, R + 1 + HB], bf16)
            nc.scalar.dma_start(aux_sb[:], aux_d.ap())
            gin_sb = constp.tile([P, KT * R], bf16)
            nc.scalar.dma_start(gin_sb[:], gin_d.ap())
            gout_sb = constp.tile([R + 1, D], bf16)
            nc.scalar.dma_start(gout_sb[:], gout_d.ap())

            # x chunk loads, alternating issue paths so early chunks share
            # SDMA bandwidth with fewer competitors
            xTs = []
            for h in range(NH):
                xT = xTp.tile([P, KT, HB], bf16)
                eng = nc.sync if h % 2 == 0 else nc.gpsimd
                eng.dma_start(
                    xT[:], x_d.ap()[:, h * KT * HB : (h + 1) * KT * HB]
                )
                xTs.append(xT)

            # PE warm-up on the memzero'd scratch (no DMA dependency): keeps
            # the PE busy while x loads so HAM un-throttles to 2.4 GHz
            warm_ps = wpsump.tile([P, NT], mybir.dt.float32)
            for w in range(N_WARM):
                nc.tensor.matmul(
                    warm_ps[:],
                    lhsT=warm_sb[:, 0:P],
                    rhs=warm_sb[:],
                    start=True,
                    stop=True,
                    skip_group_check=True,
                )

            for h in range(NH):
                xT = xTs[h]
                tT_ps = tpsump.tile([R + 1, HB], mybir.dt.float32)
                # K=1 matmul writes ones into row 16 and zeros rows 0..15
                # (start=True), which the stage-1 matmuls then accumulate into
                nc.tensor.matmul(
                    tT_ps[:],
                    lhsT=aux_sb[:, 0 : R + 1],
                    rhs=aux_sb[:, R + 1 : R + 1 + HB],
                    start=True,
                    stop=False,
                    skip_group_check=True,
                )
                for kt in range(KT):
                    nc.tensor.matmul(
                        tT_ps[0:R, :],
                        lhsT=gin_sb[:, kt * R : (kt + 1) * R],
                        rhs=xT[:, kt, :],
                        start=False,
                        stop=(kt == KT - 1),
                        skip_group_check=True,
                    )
                # t^T rows 0..15 = (x@g_in).T slice, row 16 = ones (bias row)
                tT_sb = tsbp.tile([R + 1, HB], bf16)
                nc.vector.tensor_copy(tT_sb[:], tT_ps[:])

                y_sb = ysbp.tile([P, D], bf16)
                last_chunk = h == NH - 1
                for jp in range(JT // 2):
                    # two matmuls into one 2-bank PSUM tile, one evacuation
                    y_ps = ypsump.tile([P, 2 * NT], mybir.dt.float32)
                    for half in range(2):
                        jt = jp * 2 + half
                        nc.tensor.matmul(
                            y_ps[:, half * NT : (half + 1) * NT],
                            lhsT=tT_sb[:],
                            rhs=gout_sb[:, jt * NT : (jt + 1) * NT],
                            skip_group_check=True,
                        )
                    # split PSUM->SBUF copies across DVE and ACT engines
                    if jp % 2 == 0:
                        nc.vector.tensor_copy(
                            y_sb[:, jp * 2 * NT : (jp + 1) * 2 * NT], y_ps[:]
                        )
                    else:
                        nc.scalar.copy(
                            y_sb[:, jp * 2 * NT : (jp + 1) * 2 * NT], y_ps[:]
                        )
                    # store each evacuated piece immediately, alternating DMA
                    # rings; the final chunk's tail stores in 512-col pieces
                    n_pieces = 2 if (last_chunk and jp >= 2) else 1
                    w = 2 * NT // n_pieces
                    for s in range(n_pieces):
                        c0 = jp * 2 * NT + s * w
                        eng = nc.sync if (jp + s) % 2 == 0 else nc.scalar
                        eng.dma_start(
                            y_d.ap()[h * HB : (h + 1) * HB, c0 : c0 + w],
                            y_sb[:, c0 : c0 + w],
                        )

    nc.compile()
    return nc


def _get_program():
    global _PROGRAM
    if _PROGRAM is None:
        _PROGRAM = _build_program()
    return _PROGRAM


def _host_factors(inputs):
    """Build g_in (SBUF layout) and [g_out.T; bias], all bf16, on host."""
    c = [np.asarray(inputs[f"c{i}"], dtype=np.float64) for i in range(6)]
    f = [np.asarray(inputs[f"f{i}"], dtype=np.float64) for i in range(6)]
    bias = np.asarray(inputs["bias"], dtype=np.float32)
    h = [f[i] @ c[i] for i in range(6)]  # (16,16) each
    g_out = (
        h[0][:, None, None, :] * h[1][None, :, None, :] * h[2][None, None, :, :]
    ).reshape(D, R)
    g_in = (
        h[3][:, None, None, :] * h[4][None, :, None, :] * h[5][None, None, :, :]
    ).reshape(D, R)
    # gin SBUF layout: gin_l[p, kt*R + r] = g_in[kt*128 + p, r]
    gin_l = np.ascontiguousarray(
        g_in.reshape(KT, P, R).transpose(1, 0, 2).reshape(P, KT * R)
    ).astype(ml_dtypes.bfloat16)
    goutT = np.concatenate(
        [g_out.T.astype(np.float32), bias[None, :]], axis=0
    ).astype(ml_dtypes.bfloat16)  # (17, 4096)
    aux = np.zeros((1, R + 1 + HB), dtype=ml_dtypes.bfloat16)
    aux[0, R] = 1.0
    aux[0, R + 1 :] = 1.0
    return gin_l, goutT, aux


# test-harness hooks (unused in graded path)
TRACE = False
TRACE_KW = {}
LAST_RESULTS = None


def kernel(**inputs):
    from concourse.bass_utils import run_bass_kernel_spmd

    global LAST_RESULTS
    x16 = np.asarray(inputs["x"], dtype=np.float32).astype(ml_dtypes.bfloat16)
    # pre-transpose into the device SBUF layout:
    # xall[ci, p, h, kt, b] = x[ci*NB + h*HB + b, kt*128 + p]
    x5 = x16.view(np.uint16).reshape(N_CORES, NH, HB, KT, P)
    xall = np.ascontiguousarray(x5.transpose(0, 4, 1, 3, 2)).view(
        ml_dtypes.bfloat16
    ).reshape(N_CORES, P, NH * KT * HB)
    gin_l, goutT, aux = _host_factors(inputs)
    nc = _get_program()
    in_maps = [
        {
            "xc": xall[ci],
            "gin": gin_l,
            "goutT": goutT,
            "aux": aux,
        }
        for ci in range(N_CORES)
    ]
    res = run_bass_kernel_spmd(
        nc, in_maps, core_ids=list(range(N_CORES)), trace=TRACE, **TRACE_KW
    )
    LAST_RESULTS = res
    y = np.empty((BATCH, D), dtype=np.float32)
    yv = y.view(np.uint32)
    for ci in range(N_CORES):
        # fast bf16 -> fp32 upcast: left-shift the raw bits into fp32
        r = res.results[ci]["yc"].view(np.uint16).astype(np.uint32)
        np.left_shift(r, 16, out=yv[ci * NB : (ci + 1) * NB])
    return y


if __name__ == "__main__":
    # quick smoke test with random data
    rng = np.random.default_rng(0)
    ins = {"x": rng.normal(size=(BATCH, D)).astype(np.float32)}
    for i in range(6):
        ins[f"c{i}"] = (rng.normal(size=(8, 16)) * 0.1).astype(np.float32)
        ins[f"f{i}"] = (rng.normal(size=(16, 8)) * 0.1).astype(np.float32)
    ins["bias"] = np.zeros(D, dtype=np.float32)
    y = kernel(**ins)
    print("y", y.shape, y.dtype)


# revision 16
# speedup vs baseline: 1.1487x; 1.1487x over previous
"""Trainium2 Bass kernel for the Kruskal (CP/Tucker) linear layer.

Math: the reference reconstructs W (4096x4096) from a rank-16 CP core and
Tucker factors, then computes y = x @ W.T + bias.  Because the 6D core is a
CP (Kruskal) tensor of rank 16, W itself is exactly rank 16:

    W = g_out @ g_in.T
    g_in[def, r]  = (f3@c3)[d,r] * (f4@c4)[e,r] * (f5@c5)[f,r]   (4096 x 16)
    g_out[abc, r] = (f0@c0)[a,r] * (f1@c1)[b,r] * (f2@c2)[c,r]   (4096 x 16)

so  y = (x @ g_in) @ g_out.T + bias.  The device kernel computes the two
x-dependent projections; the tiny factor-only products (g_in/g_out, ~100
KFLOP) are prepared on the host.

bf16 is used end-to-end (not fp16: the factor entries ~2e-5 and outputs
~1e-6 underflow fp16's 6e-5 min-normal, and fp16 moving operands stream at
half rate on the PE).  All device I/O is bf16, halving HBM and host<->device
traffic vs fp32.

Sharding: data-parallel over the batch (4096 rows -> 8 cores x 512). No
collectives.  x is pre-transposed on the host into the exact SBUF layout
(xT[p, h, kt, b] = x[h*128+b, kt*128+p]) so the device does plain fully
contiguous loads: an on-device xbar DMA-transpose would serialize ALL other
DMA traffic around it (Tile's deadlock guard chains every DMA before/after
a DMA_TRANSPOSE on completion semaphores) and runs at only ~224 GB/s.

Device schedule (per core, 4 pipelined 128-row chunks):
  0. scratch memzero first (no deps), tiny consts (aux/gin/gout) next on
     the scalar HWDGE ring so they are not starved behind the x stream;
     x chunk loads alternate the sync ring and the gpsimd (SWDGE) path so
     chunk 0 shares bandwidth 2-ways instead of 4-ways
  1. PE warm-up: the memzero'd scratch feeds ~14 dummy matmuls during the
     first x load so the HAM clock-gate reaches 8/8 (2.4 GHz) before real
     work; a cold PE runs every matmul at half clock
  2. stage 1: 32 accumulating matmuls  t^T(16,128) += g_in_kt.T @ xT_kt
     (one aux K=1 matmul first writes the bias ones-row 16)
  3. DVE copy PSUM -> SBUF bf16 t^T
  4. stage 2: 8 matmuls  y(128,512) = [t,1].T @ [g_out.T; bias] into 2-bank
     PSUM tiles, evacuated 1024 cols per copy, split DVE/ACT
  5. y stored bf16 per evacuated piece, alternating sync/scalar rings; the
     final chunk stores 512-col pieces to shorten the kernel tail
"""

import numpy as np
import ml_dtypes

N_CORES = 8
BATCH = 4096
D = 4096          # in/out features (16*16*16)
R = 16            # CP rank
P = 128           # partitions
NB = BATCH // N_CORES   # 512 batch rows per core
HB = 128                # batch chunk (rows per pipelined chunk)
NH = NB // HB           # 4 chunks per core
KT = D // P             # 32 feature k-tiles
NT = 512                # output column tile (one PSUM bank)
JT = D // NT            # 8 output column tiles
N_WARM = 14             # dummy matmuls to warm the PE clock-gate

_PROGRAM = None


def _build_program():
    import concourse.tile as tile
    from concourse import bacc, mybir

    nc = bacc.Bacc(
        "TRN2",
        target_bir_lowering=False,
        debug=False,
        enable_asserts=False,
        num_devices=N_CORES,
    )
    bf16 = mybir.dt.bfloat16
    # xc holds x^T in SBUF layout: xc[p, (h*KT + kt)*HB + b] = x[h*HB+b, kt*128+p]
    x_d = nc.dram_tensor("xc", (P, NH * KT * HB), bf16, kind="ExternalInput")
    gin_d = nc.dram_tensor("gin", (P, KT * R), bf16, kind="ExternalInput")
    gout_d = nc.dram_tensor("goutT", (R + 1, D), bf16, kind="ExternalInput")
    # aux row: [e16 (17 cols: zeros, col16=1), ones (HB cols)] used to write
    # the bias ones-row of t^T via a K=1 matmul
    aux_d = nc.dram_tensor("aux", (1, R + 1 + HB), bf16, kind="ExternalInput")
    y_d = nc.dram_tensor("yc", (NB, D), bf16, kind="ExternalOutput")

    with tile.TileContext(nc) as tc:
        with (
            tc.tile_pool(name="const", bufs=1) as constp,
            tc.tile_pool(name="xT", bufs=3) as xTp,
            tc.tile_pool(name="tsb", bufs=2) as tsbp,
            tc.tile_pool(name="ysb", bufs=3) as ysbp,
            tc.tile_pool(name="tpsum", bufs=2, space="PSUM") as tpsump,
            tc.tile_pool(name="ypsum", bufs=2, space="PSUM") as ypsump,
            tc.tile_pool(name="wpsum", bufs=1, space="PSUM") as wpsump,
        ):
            # scratch memzero first (no dependencies) so the PE warm-up can
            # begin as early as possible
            warm_sb = constp.tile([P, NT], bf16)
            nc.scalar.memzero(warm_sb[:])

            # tiny consts next on the scalar ring: they must not queue
            # behind the 4MB x stream (a late aux packet gates the first MM)
            aux_sb = constp.tile([1, R + 1 + HB], bf16)
            nc.scalar.dma_start(aux_sb[:], aux_d.ap())
            gin_sb = constp.tile([P, KT * R], bf16)
            nc.scalar.dma_start(gin_sb[:], gin_d.ap())
            gout_sb = constp.tile([R + 1, D], bf16)
            nc.scalar.dma_start(gout_sb[:], gout_d.ap())

            # x chunk loads, alternating issue paths so early chunks share
            # SDMA bandwidth with fewer competitors
            xTs = []
            for h in range(NH):
                xT = xTp.tile([P, KT, HB], bf16)
                eng = nc.sync if h % 2 == 0 else nc.gpsimd
                eng.dma_start(
                    xT[:], x_d.ap()[:, h * KT * HB : (h + 1) * KT * HB]
                )
                xTs.append(xT)

            # PE warm-up on the memzero'd scratch (no DMA dependency): keeps
            # the PE busy while x loads so HAM un-throttles to 2.4 GHz
            warm_ps = wpsump.tile([P, NT], mybir.dt.float32)
            for w in range(N_WARM):
                nc.tensor.matmul(
                    warm_ps[:],
                    lhsT=warm_sb[:, 0:P],
                    rhs=warm_sb[:],
                    start=True,
                    stop=True,
                    skip_group_check=True,
                )

            for h in range(NH):
                xT = xTs[h]
                tT_ps = tpsump.tile([R + 1, HB], mybir.dt.float32)
                # K=1 matmul writes ones into row 16 and zeros rows 0..15
                # (start=True), which the stage-1 matmuls then accumulate into
                nc.tensor.matmul(
                    tT_ps[:],
                    lhsT=aux_sb[:, 0 : R + 1],
                    rhs=aux_sb[:, R + 1 : R + 1 + HB],
                    start=True,
                    stop=False,
                    skip_group_check=True,
                )
                for kt in range(KT):
                    nc.tensor.matmul(
                        tT_ps[0:R, :],
                        lhsT=gin_sb[:, kt * R : (kt + 1) * R],
                        rhs=xT[:, kt, :],
                        start=False,
                        stop=(kt == KT - 1),
                        skip_group_check=True,
                    )
                # t^T rows 0..15 = (x@g_in).T slice, row 16 = ones (bias row)
                tT_sb = tsbp.tile([R + 1, HB], bf16)
                nc.vector.tensor_copy(tT_sb[:], tT_ps[:])

                y_sb = ysbp.tile([P, D], bf16)
                last_chunk = h == NH - 1
                for jp in range(JT // 2):
                    # two matmuls into one 2-bank PSUM tile, one evacuation
                    y_ps = ypsump.tile([P, 2 * NT], mybir.dt.float32)
                    for half in range(2):
                        jt = jp * 2 + half
                        nc.tensor.matmul(
                            y_ps[:, half * NT : (half + 1) * NT],
                            lhsT=tT_sb[:],
                            rhs=gout_sb[:, jt * NT : (jt + 1) * NT],
                            skip_group_check=True,
                        )
                    # split PSUM->SBUF copies across DVE and ACT engines
                    if jp % 2 == 0:
                        nc.vector.tensor_copy(
                            y_sb[:, jp * 2 * NT : (jp + 1) * 2 * NT], y_ps[:]
                        )
                    else:
                        nc.scalar.copy(
                            y_sb[:, jp * 2 * NT : (jp + 1) * 2 * NT], y_ps[:]
                        )
                    # store each evacuated piece immediately, alternating DMA
                    # rings; the final chunk's tail stores in 512-col pieces
                    n_pieces = 2 if (last_chunk and jp >= 2) else 1
                    w = 2 * NT // n_pieces
                    for s in range(n_pieces):
                        c0 = jp * 2 * NT + s * w
                        eng = nc.sync if (jp + s) % 2 == 0 else nc.scalar
                        eng.dma_start(
                            y_d.ap()[h * HB : (h + 1) * HB, c0 : c0 + w],
                            y_sb[:, c0 : c0 + w],
                        )

    nc.compile()
    return nc


def _get_program():
    global _PROGRAM
    if _PROGRAM is None:
        _PROGRAM = _build_program()
    return _PROGRAM


def _host_factors(inputs):
    """Build g_in (SBUF layout) and [g_out.T; bias], all bf16, on host."""
    c = [np.asarray(inputs[f"c{i}"], dtype=np.float64) for i in range(6)]
    f = [np.asarray(inputs[f"f{i}"], dtype=np.float64) for i in range(6)]
    bias = np.asarray(inputs["bias"], dtype=np.float32)
    h = [f[i] @ c[i] for i in range(6)]  # (16,16) each
    g_out = (
        h[0][:, None, None, :] * h[1][None, :, None, :] * h[2][None, None, :, :]
    ).reshape(D, R)
    g_in = (
        h[3][:, None, None, :] * h[4][None, :, None, :] * h[5][None, None, :, :]
    ).reshape(D, R)
    # gin SBUF layout: gin_l[p, kt*R + r] = g_in[kt*128 + p, r]
    gin_l = np.ascontiguousarray(
        g_in.reshape(KT, P, R).transpose(1, 0, 2).reshape(P, KT * R)
    ).astype(ml_dtypes.bfloat16)
    goutT = np.concatenate(
        [g_out.T.astype(np.float32), bias[None, :]], axis=0
    ).astype(ml_dtypes.bfloat16)  # (17, 4096)
    aux = np.zeros((1, R + 1 + HB), dtype=ml_dtypes.bfloat16)
    aux[0, R] = 1.0
    aux[0, R + 1 :] = 1.0
    return gin_l, goutT, aux


# test-harness hooks (unused in graded path)
TRACE = False
TRACE_KW = {}
LAST_RESULTS = None


def kernel(**inputs):
    from concourse.bass_utils import run_bass_kernel_spmd

    global LAST_RESULTS
    x16 = np.asarray(inputs["x"], dtype=np.float32).astype(ml_dtypes.bfloat16)
    # pre-transpose into the device SBUF layout:
    # xall[ci, p, h, kt, b] = x[ci*NB + h*HB + b, kt*128 + p]
    x5 = x16.view(np.uint16).reshape(N_CORES, NH, HB, KT, P)
    xall = np.ascontiguousarray(x5.transpose(0, 4, 1, 3, 2)).view(
        ml_dtypes.bfloat16
    ).reshape(N_CORES, P, NH * KT * HB)
    gin_l, goutT, aux = _host_factors(inputs)
    nc = _get_program()
    in_maps = [
        {
            "xc": xall[ci],
            "gin": gin_l,
            "goutT": goutT,
            "aux": aux,
        }
        for ci in range(N_CORES)
    ]
    res = run_bass_kernel_spmd(
        nc, in_maps, core_ids=list(range(N_CORES)), trace=TRACE, **TRACE_KW
    )
    LAST_RESULTS = res
    y = np.empty((BATCH, D), dtype=np.float32)
    yv = y.view(np.uint32)
    for ci in range(N_CORES):
        # fast bf16 -> fp32 upcast: left-shift the raw bits into fp32
        r = res.results[ci]["yc"].view(np.uint16).astype(np.uint32)
        np.left_shift(r, 16, out=yv[ci * NB : (ci + 1) * NB])
    return y


if __name__ == "__main__":
    # quick smoke test with random data
    rng = np.random.default_rng(0)
    ins = {"x": rng.normal(size=(BATCH, D)).astype(np.float32)}
    for i in range(6):
        ins[f"c{i}"] = (rng.normal(size=(8, 16)) * 0.1).astype(np.float32)
        ins[f"f{i}"] = (rng.normal(size=(16, 8)) * 0.1).astype(np.float32)
    ins["bias"] = np.zeros(D, dtype=np.float32)
    y = kernel(**ins)
    print("y", y.shape, y.dtype)
